# revision 1
# baseline (speedup 1.0000x reference)
"""Trainium2 Bass kernel for nn_DAG_72782515798738.

Math: node j (of M=1280) computes h_j = tanh(b_j + sum_{k<IN+j} W[j,k]*state_k)
over states = [x (IN=1024), h (M)], batch B=8192. Output y = sigmoid(h[HID:]).

Strategy: data-parallel over batch (8 cores x 1024 rows). Per core, the node
recurrence is solved block-by-block (10 blocks of 128 nodes) with a fixed-point
iteration h <- tanh(p + L_diag @ h) in NODE-MAJOR layout ([node, batch] tiles):
L is strictly triangular so the iteration converges superlinearly (error drops
~10x/iteration; ~9 iterations reach fp32 accuracy). Cross-block/input terms p
are accumulated in PSUM by the tensor engine (contraction 128/1024), prefetched
during the previous block's iterations. No transposes, no per-node ops.
"""
import numpy as np

import concourse.bass as bass
import concourse.mybir as mybir
from concourse.tile import TileContext
from concourse.vector_clock import ScopedClock
from concourse.bass_utils import run_bass_kernel_spmd

F32 = mybir.dt.float32
F32R = mybir.dt.float32r   # single-pass fp32 matmul (tf32-class precision, ~3x faster)
AF = mybir.ActivationFunctionType
ALU = mybir.AluOpType

IN, HID, OUT = 1024, 1024, 256
M = HID + OUT          # 1280 computed nodes
B = 8192
NCORES = 8
BC = B // NCORES       # 1024 batch rows per core
K = 128                # node block size
NB = M // K            # 10 blocks
NIT_R = 4              # float32r updates after h0 (truncation ~5e-5 < f32r noise)
NIT_F = 0              # closing full-fp32 updates (needs fp32 operand tiles; off)
HALF = BC // 2         # 512


_wsplit_ctr = [0]


class _TileContextFix(TileContext):
    """This walrus build accepts only ONE embedded sem-wait per instruction;
    split extra waits onto single-wait NOPs, and same for the exit drain."""

    def _add_instruction(self, inst):
        si = getattr(inst, "sync_info", None)
        if si is not None and si.on_wait is not None and len(si.on_wait) > 1:
            waits = list(si.on_wait)
            for w in waits[:-1]:
                _wsplit_ctr[0] += 1
                nop = mybir.InstNoOp(name=f"wsplit_{_wsplit_ctr[0]}", ins=[], outs=[])
                nop.engine = inst.engine
                nop.sync_info = mybir.SyncInfo(on_wait=[w], on_update=[])
                super()._add_instruction(nop)
            si.on_wait = waits[-1:]
        super()._add_instruction(inst)

    def _drain_and_barrier(self, tick_clock, wait_clock):
        nc = self.nc
        probe = nc.sync.nop(nofuse=True, hint="exit_wait_carrier")
        wait_clock.add_sem_waits(probe.ins, ScopedClock({None: tick_clock.global_clock}))
        si = probe.ins.sync_info
        waits = list(si.on_wait) if si is not None and si.on_wait else []
        if len(waits) > 1:
            si.on_wait = waits[:1]
            for w in waits[1:]:
                n2 = nc.sync.nop(nofuse=True, hint="exit_wait_carrier")
                if n2.ins.sync_info is None:
                    n2.ins.sync_info = mybir.SyncInfo(on_wait=[w], on_update=[])
                else:
                    n2.ins.sync_info.on_wait = [w]
        nc.sync.drain()
        nc.all_engine_barrier()
        assert self.sems is not None
        popped = nc._tile_sem_poison_stack.pop()
        assert popped is self._sem_poison
        nc.clear_and_free_semaphores(list(self.sems.allocated().values()))
        nc.all_engine_barrier()


def _build():
    nc = bass.Bass("TRN2", target_bir_lowering=False, debug=False, num_devices=NCORES)

    xT = nc.dram_tensor("xT", [IN, BC], F32R, kind="ExternalInput")
    WxT = nc.dram_tensor("WxT", [IN, M], F32R, kind="ExternalInput")
    LTd = nc.dram_tensor("LT", [M, M], F32R, kind="ExternalInput")
    bd = nc.dram_tensor("bvec", [M, 1], F32, kind="ExternalInput")
    yT = nc.dram_tensor("yT", [OUT, BC], F32, kind="ExternalOutput")

    KT = IN // 128  # 8 contraction tiles for the input matmul

    with _TileContextFix(nc) as tc:
        with (
            tc.tile_pool(name="sb", bufs=1) as sb,
            tc.tile_pool(name="ps", bufs=1, space="PSUM") as ps,
        ):
            # persistent SBUF tiles
            xt = [sb.tile([128, BC], F32R, name=f"xt{t}", tag=f"xt{t}") for t in range(KT)]
            wx = [sb.tile([128, M], F32R, name=f"wx{t}", tag=f"wx{t}") for t in range(KT)]
            # lt[i]: rows = nodes [128i,128i+128), cols = targets [128i, 1280)
            lt = [sb.tile([128, M - 128 * i], F32R, name=f"lt{i}", tag=f"lt{i}") for i in range(NB)]
            hb = [sb.tile([128, BC], F32R, name=f"h{u}", tag=f"h{u}") for u in range(NB)]
            bt = sb.tile([128, NB], F32, name="bt", tag="bt")

            # DMAs spread across engine SWDGE/HWDGE queues, ordered so block 0
            # (then 1, 2, ...) can start as early as possible: first-half xt
            # and the first wx column-block gate p0; the rest streams behind.
            nc.gpsimd.dma_start(out=bt[:], in_=bd.ap().rearrange("(u p) o -> p (u o)", p=128))
            for t in range(KT):
                nc.scalar.dma_start(out=xt[t][:, 0:HALF], in_=xT.ap()[128 * t:128 * (t + 1), 0:HALF])
                nc.gpsimd.dma_start(out=wx[t][:, 0:128], in_=WxT.ap()[128 * t:128 * (t + 1), 0:128])
            for t in range(KT):
                nc.scalar.dma_start(out=xt[t][:, HALF:], in_=xT.ap()[128 * t:128 * (t + 1), HALF:])
                nc.gpsimd.dma_start(out=wx[t][:, 128:256], in_=WxT.ap()[128 * t:128 * (t + 1), 128:256])
            nc.sync.dma_start(out=lt[0][:], in_=LTd.ap()[0:128, 0:])
            for t in range(KT):
                nc.gpsimd.dma_start(out=wx[t][:, 256:], in_=WxT.ap()[128 * t:128 * (t + 1), 256:])
            for i in range(1, NB):
                nc.sync.dma_start(
                    out=lt[i][:], in_=LTd.ap()[128 * i:128 * (i + 1), 128 * i:])

            with (
                tc.tile_pool(name="pp_pool", bufs=2, space="PSUM") as pp_pool,
                tc.tile_pool(name="lh_pool", bufs=2, space="PSUM") as lh_pool,
                tc.tile_pool(name="psb_pool", bufs=2) as psb_pool,
                tc.tile_pool(name="tin_pool", bufs=2, space="PSUM") as tin_pool,
                tc.tile_pool(name="y_pool", bufs=2) as y_pool,
            ):
                def prefetch_input(u, p_ps):
                    """input-matmul contributions to p for block u (start of group)"""
                    for h in range(2):
                        sl = slice(HALF * h, HALF * (h + 1))
                        for t in range(KT):
                            nc.tensor.matmul(
                                p_ps[:, sl],
                                wx[t][:, 128 * u:128 * (u + 1)],
                                xt[t][:, sl],
                                start=(t == 0), stop=False)

                def prefetch_cross(u, p_ps, i, last):
                    """contribution of completed block i (<u) to p of block u"""
                    for h in range(2):
                        sl = slice(HALF * h, HALF * (h + 1))
                        nc.tensor.matmul(
                            p_ps[:, sl],
                            lt[i][:, 128 * (u - i):128 * (u - i + 1)],
                            hb[i][:, sl],
                            start=False, stop=last)

                p_cur = pp_pool.tile([128, BC], F32, name="pp", tag="pp")
                prefetch_input(0, p_cur)

                for u in range(NB):
                    bcol = bt[:, u:u + 1]
                    # h0 = tanh(p + b) straight from PSUM (ACT)
                    for h in range(2):
                        sl = slice(HALF * h, HALF * (h + 1))
                        nc.scalar.activation(hb[u][:, sl], p_cur[:, sl], AF.Tanh, bias=bcol)
                    # p -> SBUF copy (DVE), frees the PSUM accumulator for prefetch
                    p_sb = psb_pool.tile([128, BC], F32, name="psb", tag="psb")
                    for h in range(2):
                        sl = slice(HALF * h, HALF * (h + 1))
                        nc.vector.tensor_copy(p_sb[:, sl], p_cur[:, sl])

                    # fixed-point iterations, two independent batch halves.
                    # NIT_R fp32r rounds converge to ~3e-5, then NIT_F full-fp32
                    # rounds restore fp32-class accuracy.
                    for k in range(NIT_R + NIT_F):
                        for h in range(2):
                            sl = slice(HALF * h, HALF * (h + 1))
                            lh = lh_pool.tile([128, HALF], F32, name="lh", tag="lh")
                            nc.tensor.matmul(
                                lh[:], lt[u][:, 0:128], hb[u][:, sl],
                                start=True, stop=True)
                            tin = tin_pool.tile([128, HALF], F32, name="tin", tag="tin")
                            nc.vector.tensor_tensor(
                                out=tin[:], in0=lh[:], in1=p_sb[:, sl], op=ALU.add)
                            nc.scalar.activation(hb[u][:, sl], tin[:], AF.Tanh, bias=bcol)

                    # prefetch next block's p (fills PE gaps in the iteration
                    # chain): input + cross from blocks <= u; the i=u pair last.
                    if u + 1 < NB:
                        p_nxt = pp_pool.tile([128, BC], F32, name="pp", tag="pp")
                        prefetch_input(u + 1, p_nxt)
                        for i in range(u):
                            prefetch_cross(u + 1, p_nxt, i, last=False)
                        prefetch_cross(u + 1, p_nxt, u, last=True)
                        p_cur = p_nxt

                    # output blocks: y = sigmoid(h), DMA out
                    if u >= NB - 2:
                        yt = y_pool.tile([128, BC], F32, name="y", tag="y")
                        for h in range(2):
                            sl = slice(HALF * h, HALF * (h + 1))
                            nc.scalar.activation(yt[:, sl], hb[u][:, sl], AF.Sigmoid)
                        r0 = 128 * (u - (NB - 2))
                        nc.gpsimd.dma_start(out=yT.ap()[r0:r0 + 128, :], in_=yt[:])
    return nc


def _enable_ldw_opt():
    """Walrus disables its LDWEIGHTS dedup by default; enabling it shaves a
    few percent here (verified correct for this kernel)."""
    import concourse.bass_utils as _bu
    if getattr(_bu.run_command, "_ldw_patched", False):
        return
    _orig = _bu.run_command

    def _patched(argv, **kw):
        try:
            argv = ["--enable-ldw-opt=true" if a == "--enable-ldw-opt=false" else a
                    for a in argv]
        except Exception:
            pass
        return _orig(argv, **kw)

    _patched._ldw_patched = True
    _bu.run_command = _patched


_nc_cache = None


def kernel(x, W, b):
    global _nc_cache
    x = np.asarray(x, dtype=np.float32)
    W = np.asarray(W, dtype=np.float32)
    b = np.asarray(b, dtype=np.float32)

    xT = np.ascontiguousarray(x.T)                       # [IN, B]
    WxT = np.ascontiguousarray(W[:, :IN].T)              # [IN, M]
    LT = np.ascontiguousarray(np.triu(W[:, IN:].T, 1))   # [M, M], LT[i,j]=W[j,IN+i], i<j
    b2 = np.ascontiguousarray(b.reshape(M, 1))

    if _nc_cache is None:
        _enable_ldw_opt()
        _nc_cache = _build()

    in_maps = [
        {"xT": np.ascontiguousarray(xT[:, c * BC:(c + 1) * BC]),
         "WxT": WxT, "LT": LT, "bvec": b2}
        for c in range(NCORES)
    ]
    res = run_bass_kernel_spmd(_nc_cache, in_maps, list(range(NCORES)))
    y = np.concatenate(
        [np.ascontiguousarray(res.results[c]["yT"].T) for c in range(NCORES)], axis=0)
    return y



# revision 9
# speedup vs baseline: 1.9237x; 1.9237x over previous
"""Trainium2 Bass kernel for nn_DAG_72782515798738.

Math: node j (of M=1280) computes h_j = tanh(b_j + sum_{k<IN+j} W[j,k]*state_k)
over states = [x (IN=1024), h (M)], batch B=8192. Output y = sigmoid(h[HID:]).

Strategy: data-parallel over batch (8 cores x 1024 rows). Per core, node-major
blocks of 128 nodes solved by fixed-point iteration h <- tanh(p + L_diag h).
All tensors bf16 (tolerance 2e-2 >> bf16-induced ~5e-3). The running PSUM bank
holds p + L h_k via delta-accumulation (bank += L (h_k - h_{k-1})), so the ACT
engine reads tanh straight from PSUM: no vector-add on the chain. Independent
"filler" matmuls (next blocks' input/cross contributions) are interleaved
between chain matmuls in PE program order so the tensor engine stays dense and
HAM-warm (2.4 GHz). Inputs stream over 4 DMA queues in compute order.
"""
import numpy as np
import ml_dtypes

import concourse.bass as bass
import concourse.mybir as mybir
from concourse.tile import TileContext
from concourse.vector_clock import ScopedClock
from concourse.bass_utils import run_bass_kernel_spmd

F32 = mybir.dt.float32
BF16 = mybir.dt.bfloat16
AF = mybir.ActivationFunctionType
ALU = mybir.AluOpType

IN, HID, OUT = 1024, 1024, 256
M = HID + OUT          # 1280 computed nodes
B = 8192
NCORES = 8
BC = B // NCORES       # 1024 batch rows per core
K = 128                # node block size
NB = M // K            # 10 blocks
KT = IN // K           # 8 contraction tiles for the input matmul
NIT = 3                # fixed-point updates after h0 (err ~13x down per update)
HALF = BC // 2         # 512
FILL = 3               # filler matmuls interleaved per chain matmul slot


_wsplit_ctr = [0]


class _TileContextFix(TileContext):
    """This walrus build accepts only ONE embedded sem-wait per instruction;
    split extra waits onto single-wait NOPs, and same for the exit drain."""

    def _add_instruction(self, inst):
        si = getattr(inst, "sync_info", None)
        if si is not None and si.on_wait is not None and len(si.on_wait) > 1:
            waits = list(si.on_wait)
            for w in waits[:-1]:
                _wsplit_ctr[0] += 1
                nop = mybir.InstNoOp(name=f"wsplit_{_wsplit_ctr[0]}", ins=[], outs=[])
                nop.engine = inst.engine
                nop.sync_info = mybir.SyncInfo(on_wait=[w], on_update=[])
                super()._add_instruction(nop)
            si.on_wait = waits[-1:]
        super()._add_instruction(inst)

    def _drain_and_barrier(self, tick_clock, wait_clock):
        nc = self.nc
        probe = nc.sync.nop(nofuse=True, hint="exit_wait_carrier")
        wait_clock.add_sem_waits(probe.ins, ScopedClock({None: tick_clock.global_clock}))
        si = probe.ins.sync_info
        waits = list(si.on_wait) if si is not None and si.on_wait else []
        if len(waits) > 1:
            si.on_wait = waits[:1]
            for w in waits[1:]:
                n2 = nc.sync.nop(nofuse=True, hint="exit_wait_carrier")
                if n2.ins.sync_info is None:
                    n2.ins.sync_info = mybir.SyncInfo(on_wait=[w], on_update=[])
                else:
                    n2.ins.sync_info.on_wait = [w]
        nc.sync.drain()
        nc.all_engine_barrier()
        assert self.sems is not None
        popped = nc._tile_sem_poison_stack.pop()
        assert popped is self._sem_poison
        nc.clear_and_free_semaphores(list(self.sems.allocated().values()))
        nc.all_engine_barrier()


def _build():
    nc = bass.Bass("TRN2", target_bir_lowering=False, debug=False, num_devices=NCORES)

    xT = nc.dram_tensor("xT", [IN, BC], BF16, kind="ExternalInput")
    WxT = nc.dram_tensor("WxT", [IN, M], BF16, kind="ExternalInput")
    LTd = nc.dram_tensor("LT", [M, M], BF16, kind="ExternalInput")
    bd = nc.dram_tensor("bvec", [M, 1], F32, kind="ExternalInput")
    yT = nc.dram_tensor("yT", [OUT, BC], F32, kind="ExternalOutput")

    with _TileContextFix(nc) as tc:
        with (
            tc.tile_pool(name="sb", bufs=1) as sb,
            tc.tile_pool(name="ps", bufs=1, space="PSUM") as ps,
        ):
            xt = [sb.tile([128, BC], BF16, name=f"xt{t}", tag=f"xt{t}") for t in range(KT)]
            wx = [sb.tile([128, M], BF16, name=f"wx{t}", tag=f"wx{t}") for t in range(KT)]
            # lt[i]: rows = nodes [128i,128i+128), cols = targets [128i, 1280)
            lt = [sb.tile([128, M - 128 * i], BF16, name=f"lt{i}", tag=f"lt{i}") for i in range(NB)]
            hb = [sb.tile([128, BC], BF16, name=f"h{u}", tag=f"h{u}") for u in range(NB)]
            tp = [sb.tile([128, BC], BF16, name=f"tp{j}", tag=f"tp{j}") for j in range(2)]
            dl = sb.tile([128, BC], BF16, name="dl", tag="dl")
            bt = sb.tile([128, NB], F32, name="bt", tag="bt")
            yt = [sb.tile([128, BC], F32, name=f"y{j}", tag=f"y{j}") for j in range(2)]
            # PSUM: 3 rotating p-banks x 2 batch halves (1 bank each)
            pb = [[ps.tile([128, HALF], F32, name=f"p{j}h{h}", tag=f"p{j}h{h}")
                   for h in range(2)] for j in range(3)]

            SL = [slice(0, HALF), slice(HALF, BC)]

            # ---- DMA schedule (per-engine issue order == per-queue order) ----
            # Only gpsimd (SW ~35GB/s), sync and scalar (HW ~100GB/s each) can
            # issue DMAs. Chunks are ordered by compute deadline.
            # gpsimd: wx first col-block (gates pre-roll), then lt diag chunks
            nc.gpsimd.dma_start(out=bt[:], in_=bd.ap().rearrange("(u p) o -> p (u o)", p=128))
            for t in range(KT):
                nc.gpsimd.dma_start(out=wx[t][:, 0:128],
                                    in_=WxT.ap()[128 * t:128 * (t + 1), 0:128])
            for i in range(1, NB):
                w = min(256, M - 128 * i)
                nc.gpsimd.dma_start(out=lt[i][:, 0:w],
                                    in_=LTd.ap()[128 * i:128 * (i + 1), 128 * i:128 * i + w])
            # scalar queue: 4 xt tiles (then free for y output later)
            for t in (0, 1, 2, 3):
                nc.scalar.dma_start(out=xt[t][:], in_=xT.ap()[128 * t:128 * (t + 1), :])
            # sync queue: everything else, deadline-ordered
            nc.sync.dma_start(out=lt[0][:, 0:256], in_=LTd.ap()[0:128, 0:256])
            for t in (4, 5, 6, 7):
                nc.sync.dma_start(out=xt[t][:], in_=xT.ap()[128 * t:128 * (t + 1), :])
            for t in range(KT):
                nc.sync.dma_start(out=wx[t][:, 128:384],
                                  in_=WxT.ap()[128 * t:128 * (t + 1), 128:384])
            nc.sync.dma_start(out=lt[0][:, 256:], in_=LTd.ap()[0:128, 256:])
            nc.sync.dma_start(out=lt[1][:, 256:], in_=LTd.ap()[128:256, 128 + 256:])
            for t in range(KT):
                nc.sync.dma_start(out=wx[t][:, 384:640],
                                  in_=WxT.ap()[128 * t:128 * (t + 1), 384:640])
            nc.sync.dma_start(out=lt[2][:, 256:], in_=LTd.ap()[256:384, 256 + 256:])
            for t in range(KT):
                nc.sync.dma_start(out=wx[t][:, 640:896],
                                  in_=WxT.ap()[128 * t:128 * (t + 1), 640:896])
            nc.sync.dma_start(out=lt[3][:, 256:], in_=LTd.ap()[384:512, 384 + 256:])
            for t in range(KT):
                nc.sync.dma_start(out=wx[t][:, 896:1152],
                                  in_=WxT.ap()[128 * t:128 * (t + 1), 896:1152])
            nc.sync.dma_start(out=lt[4][:, 256:], in_=LTd.ap()[512:640, 512 + 256:])
            nc.sync.dma_start(out=lt[5][:, 256:], in_=LTd.ap()[640:768, 640 + 256:])
            for t in range(KT):
                nc.sync.dma_start(out=wx[t][:, 1152:1280],
                                  in_=WxT.ap()[128 * t:128 * (t + 1), 1152:1280])
            for i in (6, 7):
                nc.sync.dma_start(out=lt[i][:, 256:],
                                  in_=LTd.ap()[128 * i:128 * (i + 1), 128 * i + 256:])

            # ---- PE filler machinery ----
            # queue of (target_block, emit_fn); emitted between chain matmuls
            filler = []
            started = set()   # (u, h) input-group start flags already emitted

            def enq_input(u):
                # t order roughly by DMA arrival
                for t in (0, 3, 1, 4, 5, 2, 6, 7):
                    for h in range(2):
                        def mk(u=u, t=t, h=h):
                            st = (u, h) not in started
                            started.add((u, h))
                            nc.tensor.matmul(
                                pb[u % 3][h][:, :],
                                wx[t][:, 128 * u:128 * (u + 1)], xt[t][:, SL[h]],
                                start=st, stop=False, skip_group_check=True)
                        filler.append((u, mk))

            def enq_cross(i, u):
                # contribution of finished block i to p of block u (i < u)
                for h in range(2):
                    def mk(i=i, u=u, h=h):
                        nc.tensor.matmul(
                            pb[u % 3][h][:, :],
                            lt[i][:, 128 * (u - i):128 * (u - i + 1)], hb[i][:, SL[h]],
                            start=False, stop=False, skip_group_check=True)
                    filler.append((u, mk))

            def pop_fill(n):
                for _ in range(min(n, len(filler))):
                    filler.pop(0)[1]()

            def force_pop(max_target):
                while filler and filler[0][0] <= max_target:
                    filler.pop(0)[1]()

            # ---- pre-roll: input matmuls for block 0 (gated by xt/wx DMAs) ----
            for ti, t in enumerate((0, 3, 1, 4, 5, 2, 6, 7)):
                for h in range(2):
                    nc.tensor.matmul(
                        pb[0][h][:, :], wx[t][:, 0:128], xt[t][:, SL[h]],
                        start=(ti == 0), stop=(ti == KT - 1), skip_group_check=True)
            started.update({(0, 0), (0, 1)})
            enq_input(1)
            enq_input(2)

            # ---- main loop over node blocks ----
            for u in range(NB):
                ja = u % 3
                bcol = bt[:, u:u + 1]
                if u >= 1 and u + 1 < NB:
                    for i in range(u):
                        enq_cross(i, u + 1)
                if u + 2 < NB and u >= 1:
                    enq_input(u + 2)

                # h0 = tanh(p + b) straight from PSUM
                for h in range(2):
                    nc.scalar.activation(tp[0][:, SL[h]], pb[ja][h][:, :], AF.Tanh, bias=bcol)
                    if u == 9 and h == 0:
                        # slip block-8 sigmoids into block-9 ACT wait gaps
                        nc.scalar.activation(yt[0][:, SL[0]], hb[8][:, SL[0]], AF.Sigmoid)
                        nc.scalar.dma_start(out=yT.ap()[0:128, 0:HALF], in_=yt[0][:, SL[0]])

                for k in range(1, NIT + 1):
                    for h in range(2):
                        if k >= 2:
                            # delta = h_{k-1} - h_{k-2} on the (idle) DVE
                            nc.vector.tensor_tensor(
                                out=dl[:, SL[h]], in0=tp[(k - 1) % 2][:, SL[h]],
                                in1=tp[k % 2][:, SL[h]], op=ALU.subtract)
                        pop_fill(FILL)
                        rhs = tp[0] if k == 1 else dl
                        nc.tensor.matmul(
                            pb[ja][h][:, :], lt[u][:, 0:128], rhs[:, SL[h]],
                            start=False, stop=True, skip_group_check=True)
                    dst = hb[u] if k == NIT else tp[k % 2]
                    for h in range(2):
                        nc.scalar.activation(dst[:, SL[h]], pb[ja][h][:, :], AF.Tanh, bias=bcol)
                    if u == 9 and k == 1:
                        nc.scalar.activation(yt[0][:, SL[1]], hb[8][:, SL[1]], AF.Sigmoid)
                        nc.sync.dma_start(out=yT.ap()[0:128, HALF:], in_=yt[0][:, SL[1]])

                # close out p(u+1): drain its fillers, then the i=u pair (stop)
                if u + 1 < NB:
                    force_pop(u + 1)
                    for h in range(2):
                        nc.tensor.matmul(
                            pb[(u + 1) % 3][h][:, :],
                            lt[u][:, 128:256], hb[u][:, SL[h]],
                            start=False, stop=True, skip_group_check=True)


            # final sigmoid + output DMA for block 9
            for h in range(2):
                nc.scalar.activation(yt[1][:, SL[h]], hb[9][:, SL[h]], AF.Sigmoid)
                eng = nc.scalar if h == 0 else nc.sync
                eng.dma_start(out=yT.ap()[128:256, SL[h]], in_=yt[1][:, SL[h]])
    return nc


def _enable_ldw_opt():
    """Walrus disables its LDWEIGHTS dedup by default; enabling it shaves a
    few percent here (verified correct for this kernel)."""
    import concourse.bass_utils as _bu
    if getattr(_bu.run_command, "_ldw_patched", False):
        return
    _orig = _bu.run_command

    def _patched(argv, **kw):
        try:
            argv = ["--enable-ldw-opt=true" if a == "--enable-ldw-opt=false" else a
                    for a in argv]
        except Exception:
            pass
        return _orig(argv, **kw)

    _patched._ldw_patched = True
    _bu.run_command = _patched


_nc_cache = None


def make_in_maps(x, W, b):
    bf = ml_dtypes.bfloat16
    xT = np.ascontiguousarray(x.T.astype(bf))                 # [IN, B]
    WxT = np.ascontiguousarray(W[:, :IN].T.astype(bf))        # [IN, M]
    LT = np.ascontiguousarray(np.triu(W[:, IN:].T, 1).astype(bf))  # [M, M]
    b2 = np.ascontiguousarray(b.reshape(M, 1).astype(np.float32))
    return [
        {"xT": np.ascontiguousarray(xT[:, c * BC:(c + 1) * BC]),
         "WxT": WxT, "LT": LT, "bvec": b2}
        for c in range(NCORES)
    ]


def kernel(x, W, b):
    global _nc_cache
    x = np.asarray(x, dtype=np.float32)
    W = np.asarray(W, dtype=np.float32)
    b = np.asarray(b, dtype=np.float32)

    if _nc_cache is None:
        _nc_cache = _build()

    in_maps = make_in_maps(x, W, b)
    res = run_bass_kernel_spmd(_nc_cache, in_maps, list(range(NCORES)))
    y = np.concatenate(
        [np.ascontiguousarray(res.results[c]["yT"].T) for c in range(NCORES)], axis=0)
    return y


# revision 15
# speedup vs baseline: 1.9499x; 1.0137x over previous
"""Trainium2 Bass kernel for nn_DAG_72782515798738.

Math: node j (of M=1280) computes h_j = tanh(b_j + sum_{k<IN+j} W[j,k]*state_k)
over states = [x (IN=1024), h (M)], batch B=8192. Output y = sigmoid(h[HID:]).

Strategy: data-parallel over batch (8 cores x 1024 rows). Per core, node-major
blocks of 128 nodes solved by fixed-point iteration h <- tanh(p + L_diag h).
All tensors bf16 (tolerance 2e-2 >> bf16-induced ~5e-3). The running PSUM bank
holds p + L h_k via delta-accumulation (bank += L (h_k - h_{k-1})), so the ACT
engine reads tanh straight from PSUM: no vector-add on the chain. Independent
"filler" matmuls (next blocks' input/cross contributions) are interleaved
between chain matmuls in PE program order so the tensor engine stays dense and
HAM-warm (2.4 GHz). Inputs stream over 4 DMA queues in compute order.
"""
import numpy as np
import ml_dtypes

import concourse.bass as bass
import concourse.mybir as mybir
from concourse.tile import TileContext
from concourse.vector_clock import ScopedClock
from concourse.bass_utils import run_bass_kernel_spmd

F32 = mybir.dt.float32
BF16 = mybir.dt.bfloat16
AF = mybir.ActivationFunctionType
ALU = mybir.AluOpType

IN, HID, OUT = 1024, 1024, 256
M = HID + OUT          # 1280 computed nodes
B = 8192
NCORES = 8
BC = B // NCORES       # 1024 batch rows per core
K = 128                # node block size
NB = M // K            # 10 blocks
KT = IN // K           # 8 contraction tiles for the input matmul
NIT = 2                # fixed-point updates after h0 (err ~13x down per update)
HALF = BC // 2         # 512
FILL = 6               # filler matmuls interleaved per chain matmul slot
NWARM = 40             # dummy matmuls at t=0 to lift the HAM clock gate early


_wsplit_ctr = [0]


class _TileContextFix(TileContext):
    """This walrus build accepts only ONE embedded sem-wait per instruction;
    split extra waits onto single-wait NOPs, and same for the exit drain."""

    def _add_instruction(self, inst):
        si = getattr(inst, "sync_info", None)
        if si is not None and si.on_wait is not None and len(si.on_wait) > 1:
            waits = list(si.on_wait)
            for w in waits[:-1]:
                _wsplit_ctr[0] += 1
                nop = mybir.InstNoOp(name=f"wsplit_{_wsplit_ctr[0]}", ins=[], outs=[])
                nop.engine = inst.engine
                nop.sync_info = mybir.SyncInfo(on_wait=[w], on_update=[])
                super()._add_instruction(nop)
            si.on_wait = waits[-1:]
        super()._add_instruction(inst)

    def _drain_and_barrier(self, tick_clock, wait_clock):
        nc = self.nc
        probe = nc.sync.nop(nofuse=True, hint="exit_wait_carrier")
        wait_clock.add_sem_waits(probe.ins, ScopedClock({None: tick_clock.global_clock}))
        si = probe.ins.sync_info
        waits = list(si.on_wait) if si is not None and si.on_wait else []
        if len(waits) > 1:
            si.on_wait = waits[:1]
            for w in waits[1:]:
                n2 = nc.sync.nop(nofuse=True, hint="exit_wait_carrier")
                if n2.ins.sync_info is None:
                    n2.ins.sync_info = mybir.SyncInfo(on_wait=[w], on_update=[])
                else:
                    n2.ins.sync_info.on_wait = [w]
        nc.sync.drain()
        nc.all_engine_barrier()
        assert self.sems is not None
        popped = nc._tile_sem_poison_stack.pop()
        assert popped is self._sem_poison
        nc.clear_and_free_semaphores(list(self.sems.allocated().values()))
        nc.all_engine_barrier()


def _build():
    nc = bass.Bass("TRN2", target_bir_lowering=False, debug=False, num_devices=NCORES)

    xT = nc.dram_tensor("xT", [IN, BC], BF16, kind="ExternalInput")
    WxT = nc.dram_tensor("WxT", [IN, M], BF16, kind="ExternalInput")
    LTd = nc.dram_tensor("LT", [M, M], BF16, kind="ExternalInput")
    bd = nc.dram_tensor("bvec", [M, 1], F32, kind="ExternalInput")
    yT = nc.dram_tensor("yT", [OUT, BC], F32, kind="ExternalOutput")

    with _TileContextFix(nc) as tc:
        with (
            tc.tile_pool(name="sb", bufs=1) as sb,
            tc.tile_pool(name="ps", bufs=1, space="PSUM") as ps,
        ):
            xt = [sb.tile([128, BC], BF16, name=f"xt{t}", tag=f"xt{t}") for t in range(KT)]
            wx = [sb.tile([128, M], BF16, name=f"wx{t}", tag=f"wx{t}") for t in range(KT)]
            # lt[i]: rows = nodes [128i,128i+128), cols = targets [128i, 1280)
            lt = [sb.tile([128, M - 128 * i], BF16, name=f"lt{i}", tag=f"lt{i}") for i in range(NB)]
            hb = [sb.tile([128, BC], BF16, name=f"h{u}", tag=f"h{u}") for u in range(NB)]
            tp = [sb.tile([128, BC], BF16, name=f"tp{j}", tag=f"tp{j}") for j in range(2)]
            dl = sb.tile([128, BC], BF16, name="dl", tag="dl")
            bt = sb.tile([128, NB], F32, name="bt", tag="bt")
            yt = [sb.tile([128, BC], F32, name=f"y{j}", tag=f"y{j}") for j in range(2)]
            # PSUM: 3 rotating p-banks x 2 batch halves (1 bank each)
            pb = [[ps.tile([128, HALF], F32, name=f"p{j}h{h}", tag=f"p{j}h{h}")
                   for h in range(2)] for j in range(3)]

            SL = [slice(0, HALF), slice(HALF, BC)]

            # ---- DMA schedule (per-engine issue order == per-queue order) ----
            # Only gpsimd (SW ~35GB/s), sync and scalar (HW ~100GB/s each) can
            # issue DMAs. Chunks are ordered by compute deadline.
            # gpsimd: bias + lt diag chunks (all needed late, slow queue OK)
            nc.gpsimd.dma_start(out=bt[:], in_=bd.ap().rearrange("(u p) o -> p (u o)", p=128))
            for i in range(1, NB):
                w = min(256, M - 128 * i)
                nc.gpsimd.dma_start(out=lt[i][:, 0:w],
                                    in_=LTd.ap()[128 * i:128 * (i + 1), 128 * i:128 * i + w])
            # scalar queue: pre-roll wx + xt halves, later wx chunks, y out
            for t in (0, 1, 2, 3):
                nc.scalar.dma_start(out=wx[t][:, 0:128],
                                    in_=WxT.ap()[128 * t:128 * (t + 1), 0:128])
            for t in (0, 1, 2, 3):
                nc.scalar.dma_start(out=xt[t][:], in_=xT.ap()[128 * t:128 * (t + 1), :])
            for t in range(KT):
                nc.scalar.dma_start(out=wx[t][:, 384:640],
                                    in_=WxT.ap()[128 * t:128 * (t + 1), 384:640])
            for t in range(KT):
                nc.scalar.dma_start(out=wx[t][:, 896:1152],
                                    in_=WxT.ap()[128 * t:128 * (t + 1), 896:1152])
            # sync queue: the rest, deadline-ordered
            for t in (4, 5, 6, 7):
                nc.sync.dma_start(out=wx[t][:, 0:128],
                                  in_=WxT.ap()[128 * t:128 * (t + 1), 0:128])
            nc.sync.dma_start(out=lt[0][:, 0:256], in_=LTd.ap()[0:128, 0:256])
            for t in (4, 5, 6, 7):
                nc.sync.dma_start(out=xt[t][:], in_=xT.ap()[128 * t:128 * (t + 1), :])
            for t in range(KT):
                nc.sync.dma_start(out=wx[t][:, 128:384],
                                  in_=WxT.ap()[128 * t:128 * (t + 1), 128:384])
            nc.sync.dma_start(out=lt[0][:, 256:], in_=LTd.ap()[0:128, 256:])
            nc.sync.dma_start(out=lt[1][:, 256:], in_=LTd.ap()[128:256, 128 + 256:])
            for t in range(KT):
                nc.sync.dma_start(out=wx[t][:, 640:896],
                                  in_=WxT.ap()[128 * t:128 * (t + 1), 640:896])
            nc.sync.dma_start(out=lt[2][:, 256:], in_=LTd.ap()[256:384, 256 + 256:])
            nc.sync.dma_start(out=lt[3][:, 256:], in_=LTd.ap()[384:512, 384 + 256:])
            for t in range(KT):
                nc.sync.dma_start(out=wx[t][:, 1152:1280],
                                  in_=WxT.ap()[128 * t:128 * (t + 1), 1152:1280])
            nc.sync.dma_start(out=lt[4][:, 256:], in_=LTd.ap()[512:640, 512 + 256:])
            nc.sync.dma_start(out=lt[5][:, 256:], in_=LTd.ap()[640:768, 640 + 256:])
            for i in (6, 7):
                nc.sync.dma_start(out=lt[i][:, 256:],
                                  in_=LTd.ap()[128 * i:128 * (i + 1), 128 * i + 256:])

            # ---- PE filler machinery ----
            # queue of (target_block, emit_fn); emitted between chain matmuls
            filler = []
            started = set()   # (u, h) input-group start flags already emitted

            def enq_input(u):
                # t order roughly by DMA arrival
                for t in (0, 3, 1, 4, 5, 2, 6, 7):
                    for h in range(2):
                        def mk(u=u, t=t, h=h):
                            st = (u, h) not in started
                            started.add((u, h))
                            nc.tensor.matmul(
                                pb[u % 3][h][:, :],
                                wx[t][:, 128 * u:128 * (u + 1)], xt[t][:, SL[h]],
                                start=st, stop=False, skip_group_check=True)
                        filler.append((u, mk))

            def enq_cross(i, u):
                # contribution of finished block i to p of block u (i < u)
                for h in range(2):
                    def mk(i=i, u=u, h=h):
                        nc.tensor.matmul(
                            pb[u % 3][h][:, :],
                            lt[i][:, 128 * (u - i):128 * (u - i + 1)], hb[i][:, SL[h]],
                            start=False, stop=False, skip_group_check=True)
                    filler.append((u, mk))

            def pop_fill(n):
                for _ in range(min(n, len(filler))):
                    filler.pop(0)[1]()

            def force_pop(max_target):
                while filler and filler[0][0] <= max_target:
                    filler.pop(0)[1]()

            # ---- HAM warmup: dummy matmuls on uninitialized scratch keep the
            # PE busy from t~6us so the clock gate lifts before real work ----
            scr_w = sb.tile([128, 128], BF16, name="scrw", tag="scrw")
            scr_x = sb.tile([128, HALF], BF16, name="scrx", tag="scrx")
            scr_p = ps.tile([128, HALF], F32, name="scrp", tag="scrp")
            nc.vector.memset(scr_w[:, :], 0.0)
            nc.vector.memset(scr_x[:, 0:128], 0.0)
            for w in range(NWARM):
                nc.tensor.matmul(scr_p[:, 0:128], scr_w[:, :], scr_x[:, 0:128],
                                 start=True, stop=True, skip_group_check=True)

            # ---- pre-roll: input matmuls for block 0 (gated by xt/wx DMAs) ----
            for ti, t in enumerate((0, 3, 1, 4, 5, 2, 6, 7)):
                for h in range(2):
                    nc.tensor.matmul(
                        pb[0][h][:, :], wx[t][:, 0:128], xt[t][:, SL[h]],
                        start=(ti == 0), stop=(ti == KT - 1), skip_group_check=True)
            started.update({(0, 0), (0, 1)})
            enq_input(1)
            enq_input(2)

            # ---- main loop over node blocks ----
            for u in range(NB):
                ja = u % 3
                bcol = bt[:, u:u + 1]
                if u >= 1 and u + 1 < NB:
                    for i in range(u):
                        enq_cross(i, u + 1)
                if u + 2 < NB and u >= 1:
                    enq_input(u + 2)

                # h0 = tanh(p + b) straight from PSUM
                for h in range(2):
                    nc.scalar.activation(tp[0][:, SL[h]], pb[ja][h][:, :], AF.Tanh, bias=bcol)
                    if u == 9 and h == 0:
                        # slip block-8 sigmoids into block-9 ACT wait gaps
                        nc.scalar.activation(yt[0][:, SL[0]], hb[8][:, SL[0]], AF.Sigmoid)
                        nc.scalar.dma_start(out=yT.ap()[0:128, 0:HALF], in_=yt[0][:, SL[0]])

                for k in range(1, NIT + 1):
                    for h in range(2):
                        if k >= 2:
                            # delta = h_{k-1} - h_{k-2} on the (idle) DVE
                            nc.vector.tensor_tensor(
                                out=dl[:, SL[h]], in0=tp[(k - 1) % 2][:, SL[h]],
                                in1=tp[k % 2][:, SL[h]], op=ALU.subtract)
                        pop_fill(FILL)
                        rhs = tp[0] if k == 1 else dl
                        nc.tensor.matmul(
                            pb[ja][h][:, :], lt[u][:, 0:128], rhs[:, SL[h]],
                            start=False, stop=True, skip_group_check=True)
                    dst = hb[u] if k == NIT else tp[k % 2]
                    for h in range(2):
                        nc.scalar.activation(dst[:, SL[h]], pb[ja][h][:, :], AF.Tanh, bias=bcol)
                    if u == 9 and k == 1:
                        nc.scalar.activation(yt[0][:, SL[1]], hb[8][:, SL[1]], AF.Sigmoid)
                        nc.sync.dma_start(out=yT.ap()[0:128, HALF:], in_=yt[0][:, SL[1]])

                # close out p(u+1): the i=u pair first (it gates A0(u+1) and
                # only waits on A_NIT here), then drain remaining fillers
                if u + 1 < NB:
                    for h in range(2):
                        nc.tensor.matmul(
                            pb[(u + 1) % 3][h][:, :],
                            lt[u][:, 128:256], hb[u][:, SL[h]],
                            start=False, stop=True, skip_group_check=True)
                    force_pop(u + 1)


            # final sigmoid + output DMA for block 9
            for h in range(2):
                nc.scalar.activation(yt[1][:, SL[h]], hb[9][:, SL[h]], AF.Sigmoid)
                eng = nc.scalar if h == 0 else nc.sync
                eng.dma_start(out=yT.ap()[128:256, SL[h]], in_=yt[1][:, SL[h]])
    return nc


def _enable_ldw_opt():
    """Walrus disables its LDWEIGHTS dedup by default; enabling it shaves a
    few percent here (verified correct for this kernel)."""
    import concourse.bass_utils as _bu
    if getattr(_bu.run_command, "_ldw_patched", False):
        return
    _orig = _bu.run_command

    def _patched(argv, **kw):
        try:
            argv = ["--enable-ldw-opt=true" if a == "--enable-ldw-opt=false" else a
                    for a in argv]
        except Exception:
            pass
        return _orig(argv, **kw)

    _patched._ldw_patched = True
    _bu.run_command = _patched


_nc_cache = None


def make_in_maps(x, W, b):
    bf = ml_dtypes.bfloat16
    xT = np.ascontiguousarray(x.T.astype(bf))                 # [IN, B]
    WxT = np.ascontiguousarray(W[:, :IN].T.astype(bf))        # [IN, M]
    LT = np.ascontiguousarray(np.triu(W[:, IN:].T, 1).astype(bf))  # [M, M]
    b2 = np.ascontiguousarray(b.reshape(M, 1).astype(np.float32))
    return [
        {"xT": np.ascontiguousarray(xT[:, c * BC:(c + 1) * BC]),
         "WxT": WxT, "LT": LT, "bvec": b2}
        for c in range(NCORES)
    ]


def kernel(x, W, b):
    global _nc_cache
    x = np.asarray(x, dtype=np.float32)
    W = np.asarray(W, dtype=np.float32)
    b = np.asarray(b, dtype=np.float32)

    if _nc_cache is None:
        _nc_cache = _build()

    in_maps = make_in_maps(x, W, b)
    res = run_bass_kernel_spmd(_nc_cache, in_maps, list(range(NCORES)))
    y = np.concatenate(
        [np.ascontiguousarray(res.results[c]["yT"].T) for c in range(NCORES)], axis=0)
    return y
